# revision 52
# baseline (speedup 1.0000x reference)
"""Trainium2 Bass kernel for nn_Chf_Likelihood_Loss.

Reference computes, for B=8 density maps of H=W=64:
    loss = mean_b sum_ij |CHF_ij(out_b) - CHF_ij(gt_b)|^2
where CHF_ij(m) = sum_n exp(I*(f_j*x_n + f_i*y_n)) m_n over the N=4096 pixels
and (f_i) are 2S=60 frequencies.

Algebraic reductions that make this tiny:
  1. CHF is linear in the map, so CHF(out) - CHF(gt) = CHF(out - gt).
  2. The angle f_j*x_w + f_i*y_h is separable, so the [60,60,4096] template
     contraction factorizes into skinny matmuls against [64,60] cos/sin
     factor matrices:
        A[i,w] = sum_h cos(f_i y_h) D[h,w],  Bm[i,w] = sum_h sin(f_i y_h) D[h,w]
        R = A@CxT - Bm@SxT,  I = A@SxT + Bm@CxT      (CxT[w,j] = cos(f_j x_w))
        loss_b = sum(R^2 + I^2)
  3. The subtraction in (1) is folded into the stage-1 matmul: stack d on
     partitions 0:64 and g on 64:128, with rhs = [trig; -trig], so the K=128
     contraction emits the transform of (d - g) directly.

Sharding: data-parallel over batch, one map per NeuronCore (B == 8 == n_cores).
Per core, everything arrives as ONE [128, 305] DMA:
    blob rows 0:64  = [ d | CT | ST | -ST | CT | ones ]
    blob rows 64:128= [ g |-CT |-ST |  *  |  *  |  *  ]   (* = unused)
then: MM1 (K=128) -> s1 copy -> MM2 x2 -> square -> ones-matmul partition sum
-> scalar reduce -> one-descriptor [1,1] DMA out. Host sums the 8 partials
and divides by B.

Raw bacc (no TileContext): the Tile tail drain/EVSEM butterfly costs ~15us,
an order of magnitude more than this kernel's work, so semaphores are manual.
"""

import numpy as np

import concourse.bacc as bacc
import concourse.bass as bass
from concourse import mybir

B, H, W = 8, 64, 64
CHF_STEP = 30
CHF_TIK = 0.01
SAMPLE_STEP = 1.0
SCALE = 1.0
S2 = 2 * CHF_STEP  # 60 frequencies
N_CORES = 8

# Input arrives as two tensors so the stage-1 matmul can start as soon as the
# first (smaller) DMA lands:
#   blobA [128, 184]: [ map | T1 ]  with T1 rows 0:64 = [CT|ST], rows 64:128 = [-CT|-ST]
#   blobB [128, 121]: [ T2 | ones ] with T2 rows 0:64 = [CT|ST], rows 64:128 = [-ST|CT]
_C_M = 0
_C_T1 = W                 # stage-1 rhs (K=128, N=120) in blobA
_CA_END = W + 2 * S2      # 184
_C_T2 = 0                 # stage-2 rhs (K=128, N=120) in blobB
_C_ONE = 2 * S2           # ones column (partition-sum lhsT, K=60) in blobB
_CB_END = _C_ONE + 1      # 121

_F32 = mybir.dt.float32
# fp16 operands: single-pass PE matmuls (fp32 runs dual-pass LOW_HIGH), half
# the DMA bytes, fp32 PSUM accumulation. End-to-end rel err ~2e-5 (simulated).
_F16 = mybir.dt.float16


def _make_blob_consts() -> tuple[np.ndarray, np.ndarray]:
    """(t1, blobB) constant blocks.

    t1    [128, 120] = [[CT|ST]; [-CT|-ST]]
    blobB [128, 121] = [[CT|ST|ones]; [-ST|CT|ones]]
    with CT[w, j] = cos(f_j * x_w). x_axis == y_axis here (H == W, same
    sampling), so the same matrix serves the stage-1 (y) and stage-2 (x)
    contractions.
    """
    half = SAMPLE_STEP / 2
    x_axis = np.linspace(half, W * SAMPLE_STEP - half, W).astype(np.float32)
    freqs = (np.arange(-CHF_STEP, CHF_STEP) * CHF_TIK).astype(np.float32)
    ang = np.outer(x_axis, freqs).astype(np.float32)  # [W, S2]
    ct = np.cos(ang).astype(np.float32)
    st = np.sin(ang).astype(np.float32)
    ones = np.ones((W, 1), dtype=np.float32)
    t1 = np.concatenate(
        [np.concatenate([ct, st], axis=1), np.concatenate([-ct, -st], axis=1)], axis=0
    )
    blob_b = np.concatenate(
        [
            np.concatenate([ct, st, ones], axis=1),
            np.concatenate([-st, ct, ones], axis=1),
        ],
        axis=0,
    )
    return t1.astype(np.float16), np.ascontiguousarray(blob_b.astype(np.float16))


def _build_bass() -> bass.Bass:
    # Strip removable fixed overheads (~4.3us measured): the const-AP memsets
    # emitted in Bass.__init__ (this kernel never uses const APs) and the
    # bass-level all-engine barriers (init + Block exit). The data-dependency
    # semaphore chain below fully orders the kernel, and walrus's own NEFF
    # epilogue still drains + barriers every engine before the semaphore wipe.
    orig_barrier = bass.Bass.all_engine_barrier
    orig_memset = bass.BassGpSimd.memset
    bass.Bass.all_engine_barrier = lambda self, *a, **k: None
    bass.BassGpSimd.memset = lambda self, *a, **k: None
    try:
        nc = _build_bass_inner()
    finally:
        bass.Bass.all_engine_barrier = orig_barrier
        bass.BassGpSimd.memset = orig_memset
    return nc


def _build_bass_inner() -> bass.Bass:
    nc = bacc.Bacc("TRN2", target_bir_lowering=False, debug=False, num_devices=N_CORES)

    blob_a_in = nc.dram_tensor("blobA", [2 * H, _CA_END], _F16, kind="ExternalInput")
    blob_b_in = nc.dram_tensor("blobB", [2 * H, _CB_END], _F16, kind="ExternalInput")
    o_out = nc.dram_tensor("o", [1, 1], _F32, kind="ExternalOutput")

    with (
        nc.sbuf_tensor([2 * H, _CA_END], _F16) as blob_a,
        nc.sbuf_tensor([2 * H, _CB_END], _F16) as blob_b,
        nc.sbuf_tensor([2 * W, S2], _F16) as s1,
        nc.sbuf_tensor([S2, 2 * S2], _F16) as c2,
        nc.sbuf_tensor([S2, 2 * S2], _F16) as sq,
        nc.sbuf_tensor([1, 1], _F32) as acc,
        nc.psum_tensor([W, 2 * S2], _F32) as ps1,
        nc.psum_tensor([S2, 2 * S2], _F32) as ps2,
        nc.psum_tensor([1, 2 * S2], _F32) as ps3,
        nc.semaphore("dma_a") as dma_a_sem,
        nc.semaphore("dma_b") as dma_b_sem,
        nc.semaphore("pe") as pe_sem,
        nc.semaphore("dve") as dve_sem,
        nc.semaphore("dma_out") as dma_out_sem,
        nc.Block() as block,
    ):

        # The input DMAs issue from the Scalar/ACT sequencer (the second HWDGE
        # ring): it clears the walrus prologue ~0.9us before Sync, whose
        # prologue ends in a long DGE drain. A (map+T1) lands first and
        # unblocks MM1 while B (T2+ones) is still in flight.
        @block.scalar
        def _(scalar):
            scalar.dma_start(out=blob_a[:], in_=blob_a_in[:]).then_inc(dma_a_sem, 16)
            scalar.dma_start(out=blob_b[:], in_=blob_b_in[:]).then_inc(dma_b_sem, 16)
            scalar.wait_ge(dve_sem, 5)
            # [1,1] output: one descriptor; a [60,1] partition-strided store
            # costs ~7us in per-descriptor HBM-write latency, and the warm ACT
            # ring issues ~100ns faster than a cold first DMA on Sync's ring.
            scalar.dma_start(out=o_out[:], in_=acc[:]).then_inc(dma_out_sem, 16)
            # Hold the queue open until the output lands in HBM (NRT reads the
            # buffer as soon as all queues retire). Semaphores need no manual
            # reset: walrus's NEFF epilogue wipes the whole sem file for
            # re-execution.
            scalar.wait_ge(dma_out_sem, 16)

        @block.tensor
        def _(tensor):
            # Stage 1 with the subtraction folded in:
            # ps1 = [d;g].T @ [trig;-trig] -> [W, 120] = [A^T | Bm^T] of (d-g)
            tensor.wait_ge(dma_a_sem, 16)
            nc.tensor.matmul(
                ps1[:],
                blob_a[:, _C_M : _C_M + W],
                blob_a[:, _C_T1 : _C_T1 + 2 * S2],
                start=True,
                stop=True,
            ).then_inc(pe_sem, 1)
            # blobB lands long before MM1 finishes; absorbing its wait here
            # keeps MM2 itself down to the single dve wait.
            tensor.wait_ge(dma_b_sem, 16)
            # Stage 2 as ONE K=128 matmul: lhsT = [A^T; Bm^T] stacked on
            # partitions, rhs = [[CT|ST]; [-ST|CT]], so
            # ps2 = [R | I] = A@[CT|ST] + Bm@[-ST|CT]
            tensor.wait_ge(dve_sem, 2)
            nc.tensor.matmul(
                ps2[:],
                s1[:],
                blob_b[:, _C_T2 : _C_T2 + 2 * S2],
                start=True,
                stop=True,
            ).then_inc(pe_sem, 1)
            # Partition sum of the squares: ps3 = ones.T @ sq -> [1, 120]
            tensor.wait_ge(dve_sem, 4)
            nc.tensor.matmul(
                ps3[:],
                blob_b[0:S2, _C_ONE : _C_ONE + 1],
                sq[:],
                start=True,
                stop=True,
            ).then_inc(pe_sem, 1)

        @block.vector
        def _(vector):
            # Matmul lhsT must live in SBUF; re-layout [A^T | Bm^T] [64,120]
            # into [A^T; Bm^T] [128,60] while bouncing ps1 out of PSUM.
            vector.wait_ge(pe_sem, 1)
            nc.vector.tensor_copy(s1[0:W, :], ps1[:, 0:S2]).then_inc(dve_sem, 1)
            nc.vector.tensor_copy(s1[W : 2 * W, :], ps1[:, S2 : 2 * S2]).then_inc(
                dve_sem, 1
            )
            # Square: sq = [R|I]^2 elementwise (one PSUM operand max per op)
            vector.wait_ge(pe_sem, 2)
            nc.vector.tensor_copy(c2[:], ps2[:]).then_inc(dve_sem, 1)
            vector.wait_ge(dve_sem, 3)
            nc.vector.tensor_mul(sq[:], c2[:], c2[:]).then_inc(dve_sem, 1)
            # Final free-axis reduce of the [1, 120] partition sums. ([1,1]
            # output: a [1,120] f32 store costs +330ns of DMA issue time.)
            vector.wait_ge(pe_sem, 3)
            nc.vector.reduce_sum(acc[:], ps3[:], axis=mybir.AxisListType.X).then_inc(
                dve_sem, 1
            )

    nc.compile()
    return nc


def _run(inputs: dict, trace: bool = False):
    from concourse.bass_utils import run_bass_kernel_spmd

    dnn = np.ascontiguousarray(np.asarray(inputs["dnn_output"], dtype=np.float32))
    gt = np.ascontiguousarray(np.asarray(inputs["gt_density_map"], dtype=np.float32))
    assert dnn.shape == (B, H, W) and gt.shape == (B, H, W)

    t1, blob_b = _make_blob_consts()
    nc = _build_bass()
    in_maps = []
    for b in range(B):
        maps = np.concatenate(
            [dnn[b].astype(np.float16), gt[b].astype(np.float16)], axis=0
        )  # [128, 64]
        blob_a = np.ascontiguousarray(np.concatenate([maps, t1], axis=1))
        in_maps.append({"blobA": blob_a, "blobB": blob_b})
    res = run_bass_kernel_spmd(nc, in_maps, list(range(N_CORES)), trace=trace)
    total = np.sum(
        np.stack([res.results[b]["o"] for b in range(B)]), dtype=np.float64
    )
    loss = np.float32(total / B * SCALE)
    return np.asarray(loss, dtype=np.float32), res  # sum covers cores AND the 120 partials


def kernel(**inputs) -> np.ndarray:
    loss, _ = _run(inputs, trace=False)
    return loss


# revision 53
# speedup vs baseline: 1.0138x; 1.0138x over previous
"""Trainium2 Bass kernel for nn_Chf_Likelihood_Loss.

Reference computes, for B=8 density maps of H=W=64:
    loss = mean_b sum_ij |CHF_ij(out_b) - CHF_ij(gt_b)|^2
where CHF_ij(m) = sum_n exp(I*(f_j*x_n + f_i*y_n)) m_n over the N=4096 pixels
and (f_i) are 2S=60 frequencies.

Algebraic reductions that make this tiny:
  1. CHF is linear in the map, so CHF(out) - CHF(gt) = CHF(out - gt).
  2. The angle f_j*x_w + f_i*y_h is separable, so the [60,60,4096] template
     contraction factorizes into skinny matmuls against [64,60] cos/sin
     factor matrices:
        A[i,w] = sum_h cos(f_i y_h) D[h,w],  Bm[i,w] = sum_h sin(f_i y_h) D[h,w]
        R = A@CxT - Bm@SxT,  I = A@SxT + Bm@CxT      (CxT[w,j] = cos(f_j x_w))
        loss_b = sum(R^2 + I^2)
  3. The subtraction in (1) is folded into the stage-1 matmul: stack d on
     partitions 0:64 and g on 64:128, with rhs = [trig; -trig], so the K=128
     contraction emits the transform of (d - g) directly.

Sharding: data-parallel over batch, one map per NeuronCore (B == 8 == n_cores).
Per core, everything arrives as ONE [128, 305] DMA:
    blob rows 0:64  = [ d | CT | ST | -ST | CT | ones ]
    blob rows 64:128= [ g |-CT |-ST |  *  |  *  |  *  ]   (* = unused)
then: MM1 (K=128) -> s1 copy -> MM2 x2 -> square -> ones-matmul partition sum
-> scalar reduce -> one-descriptor [1,1] DMA out. Host sums the 8 partials
and divides by B.

Raw bacc (no TileContext): the Tile tail drain/EVSEM butterfly costs ~15us,
an order of magnitude more than this kernel's work, so semaphores are manual.
"""

import numpy as np

import concourse.bacc as bacc
import concourse.bass as bass
from concourse import mybir

B, H, W = 8, 64, 64
CHF_STEP = 30
CHF_TIK = 0.01
SAMPLE_STEP = 1.0
SCALE = 1.0
S2 = 2 * CHF_STEP  # 60 frequencies
N_CORES = 8

# Input arrives as two tensors so the stage-1 matmul can start as soon as the
# first (smaller) DMA lands:
#   blobA [128, 184]: [ map | T1 ]  with T1 rows 0:64 = [CT|ST], rows 64:128 = [-CT|-ST]
#   blobB [128, 121]: [ T2 | ones ] with T2 rows 0:64 = [CT|ST], rows 64:128 = [-ST|CT]
_C_M = 0
_C_T1 = W                 # stage-1 rhs (K=128, N=120) in blobA
_CA_END = W + 2 * S2      # 184
_C_T2 = 0                 # stage-2 rhs (K=128, N=120) in blobB
_C_ONE = 2 * S2           # ones column (partition-sum lhsT, K=60) in blobB
_CB_END = _C_ONE + 1      # 121

_F32 = mybir.dt.float32
# fp16 operands: single-pass PE matmuls (fp32 runs dual-pass LOW_HIGH), half
# the DMA bytes, fp32 PSUM accumulation. End-to-end rel err ~2e-5 (simulated).
_F16 = mybir.dt.float16


def _make_blob_consts() -> tuple[np.ndarray, np.ndarray]:
    """(t1, blobB) constant blocks.

    t1    [128, 120] = [[CT|ST]; [-CT|-ST]]
    blobB [128, 121] = [[CT|ST|ones]; [-ST|CT|ones]]
    with CT[w, j] = cos(f_j * x_w). x_axis == y_axis here (H == W, same
    sampling), so the same matrix serves the stage-1 (y) and stage-2 (x)
    contractions.
    """
    half = SAMPLE_STEP / 2
    x_axis = np.linspace(half, W * SAMPLE_STEP - half, W).astype(np.float32)
    freqs = (np.arange(-CHF_STEP, CHF_STEP) * CHF_TIK).astype(np.float32)
    ang = np.outer(x_axis, freqs).astype(np.float32)  # [W, S2]
    ct = np.cos(ang).astype(np.float32)
    st = np.sin(ang).astype(np.float32)
    ones = np.ones((W, 1), dtype=np.float32)
    t1 = np.concatenate(
        [np.concatenate([ct, st], axis=1), np.concatenate([-ct, -st], axis=1)], axis=0
    )
    blob_b = np.concatenate(
        [
            np.concatenate([ct, st, ones], axis=1),
            np.concatenate([-st, ct, ones], axis=1),
        ],
        axis=0,
    )
    return t1.astype(np.float16), np.ascontiguousarray(blob_b.astype(np.float16))


def _build_bass() -> bass.Bass:
    # Strip removable fixed overheads (~4.3us measured): the const-AP memsets
    # emitted in Bass.__init__ (this kernel never uses const APs) and the
    # bass-level all-engine barriers (init + Block exit). The data-dependency
    # semaphore chain below fully orders the kernel, and walrus's own NEFF
    # epilogue still drains + barriers every engine before the semaphore wipe.
    orig_barrier = bass.Bass.all_engine_barrier
    orig_memset = bass.BassGpSimd.memset
    bass.Bass.all_engine_barrier = lambda self, *a, **k: None
    bass.BassGpSimd.memset = lambda self, *a, **k: None
    try:
        nc = _build_bass_inner()
    finally:
        bass.Bass.all_engine_barrier = orig_barrier
        bass.BassGpSimd.memset = orig_memset
    return nc


def _build_bass_inner() -> bass.Bass:
    nc = bacc.Bacc("TRN2", target_bir_lowering=False, debug=False, num_devices=N_CORES)

    blob_a_in = nc.dram_tensor("blobA", [2 * H, _CA_END], _F16, kind="ExternalInput")
    blob_b_in = nc.dram_tensor("blobB", [2 * H, _CB_END], _F16, kind="ExternalInput")
    o_out = nc.dram_tensor("o", [1, 1], _F32, kind="ExternalOutput")

    with (
        nc.sbuf_tensor([2 * H, _CA_END], _F16) as blob_a,
        nc.sbuf_tensor([2 * H, _CB_END], _F16) as blob_b,
        nc.sbuf_tensor([2 * W, S2], _F16) as s1,
        nc.sbuf_tensor([S2, 2 * S2], _F16) as c2,
        nc.sbuf_tensor([S2, 2 * S2], _F16) as sq,
        nc.sbuf_tensor([1, 1], _F32) as acc,
        nc.psum_tensor([W, 2 * S2], _F32) as ps1,
        nc.psum_tensor([S2, 2 * S2], _F32) as ps2,
        nc.psum_tensor([1, 2 * S2], _F32) as ps3,
        nc.semaphore("dma_a") as dma_a_sem,
        nc.semaphore("dma_b") as dma_b_sem,
        nc.semaphore("pe") as pe_sem,
        nc.semaphore("dve") as dve_sem,
        nc.semaphore("dma_out") as dma_out_sem,
        nc.Block() as block,
    ):

        # The input DMAs issue from the Scalar/ACT sequencer (the second HWDGE
        # ring): it clears the walrus prologue ~0.9us before Sync, whose
        # prologue ends in a long DGE drain. A (map+T1) lands first and
        # unblocks MM1 while B (T2+ones) is still in flight.
        @block.scalar
        def _(scalar):
            scalar.dma_start(out=blob_a[:], in_=blob_a_in[:]).then_inc(dma_a_sem, 16)
            scalar.dma_start(out=blob_b[:], in_=blob_b_in[:]).then_inc(dma_b_sem, 16)

        @block.sync
        def _(sync):
            sync.wait_ge(dve_sem, 5)
            # [1,1] output: one descriptor; a [60,1] partition-strided store
            # costs ~7us in per-descriptor HBM-write latency. (Sync's ring
            # issues this in ~0.7us; a third DMA on the ACT ring took ~1.1us.)
            sync.dma_start(out=o_out[:], in_=acc[:]).then_inc(dma_out_sem, 16)
            # Hold the queue open until the output lands in HBM (NRT reads the
            # buffer as soon as all queues retire). Semaphores need no manual
            # reset: walrus's NEFF epilogue wipes the whole sem file for
            # re-execution.
            sync.wait_ge(dma_out_sem, 16)

        @block.tensor
        def _(tensor):
            # Stage 1 with the subtraction folded in:
            # ps1 = [d;g].T @ [trig;-trig] -> [W, 120] = [A^T | Bm^T] of (d-g)
            tensor.wait_ge(dma_a_sem, 16)
            nc.tensor.matmul(
                ps1[:],
                blob_a[:, _C_M : _C_M + W],
                blob_a[:, _C_T1 : _C_T1 + 2 * S2],
                start=True,
                stop=True,
            ).then_inc(pe_sem, 1)
            # blobB lands long before MM1 finishes; absorbing its wait here
            # keeps MM2 itself down to the single dve wait.
            tensor.wait_ge(dma_b_sem, 16)
            # Stage 2 as ONE K=128 matmul: lhsT = [A^T; Bm^T] stacked on
            # partitions, rhs = [[CT|ST]; [-ST|CT]], so
            # ps2 = [R | I] = A@[CT|ST] + Bm@[-ST|CT]
            tensor.wait_ge(dve_sem, 2)
            nc.tensor.matmul(
                ps2[:],
                s1[:],
                blob_b[:, _C_T2 : _C_T2 + 2 * S2],
                start=True,
                stop=True,
            ).then_inc(pe_sem, 1)
            # Partition sum of the squares: ps3 = ones.T @ sq -> [1, 120]
            tensor.wait_ge(dve_sem, 4)
            nc.tensor.matmul(
                ps3[:],
                blob_b[0:S2, _C_ONE : _C_ONE + 1],
                sq[:],
                start=True,
                stop=True,
            ).then_inc(pe_sem, 1)

        @block.vector
        def _(vector):
            # Matmul lhsT must live in SBUF; re-layout [A^T | Bm^T] [64,120]
            # into [A^T; Bm^T] [128,60] while bouncing ps1 out of PSUM.
            vector.wait_ge(pe_sem, 1)
            nc.vector.tensor_copy(s1[0:W, :], ps1[:, 0:S2]).then_inc(dve_sem, 1)
            nc.vector.tensor_copy(s1[W : 2 * W, :], ps1[:, S2 : 2 * S2]).then_inc(
                dve_sem, 1
            )
            # Square: sq = [R|I]^2 elementwise (one PSUM operand max per op)
            vector.wait_ge(pe_sem, 2)
            nc.vector.tensor_copy(c2[:], ps2[:]).then_inc(dve_sem, 1)
            vector.wait_ge(dve_sem, 3)
            nc.vector.tensor_mul(sq[:], c2[:], c2[:]).then_inc(dve_sem, 1)
            # Final free-axis reduce of the [1, 120] partition sums. ([1,1]
            # output: a [1,120] f32 store costs +330ns of DMA issue time.)
            vector.wait_ge(pe_sem, 3)
            nc.vector.reduce_sum(acc[:], ps3[:], axis=mybir.AxisListType.X).then_inc(
                dve_sem, 1
            )

    nc.compile()
    return nc


def _run(inputs: dict, trace: bool = False):
    from concourse.bass_utils import run_bass_kernel_spmd

    dnn = np.ascontiguousarray(np.asarray(inputs["dnn_output"], dtype=np.float32))
    gt = np.ascontiguousarray(np.asarray(inputs["gt_density_map"], dtype=np.float32))
    assert dnn.shape == (B, H, W) and gt.shape == (B, H, W)

    t1, blob_b = _make_blob_consts()
    nc = _build_bass()
    in_maps = []
    for b in range(B):
        maps = np.concatenate(
            [dnn[b].astype(np.float16), gt[b].astype(np.float16)], axis=0
        )  # [128, 64]
        blob_a = np.ascontiguousarray(np.concatenate([maps, t1], axis=1))
        in_maps.append({"blobA": blob_a, "blobB": blob_b})
    res = run_bass_kernel_spmd(nc, in_maps, list(range(N_CORES)), trace=trace)
    total = np.sum(
        np.stack([res.results[b]["o"] for b in range(B)]), dtype=np.float64
    )
    loss = np.float32(total / B * SCALE)
    return np.asarray(loss, dtype=np.float32), res  # sum covers cores AND the 120 partials


def kernel(**inputs) -> np.ndarray:
    loss, _ = _run(inputs, trace=False)
    return loss


# revision 55
# speedup vs baseline: 1.0183x; 1.0044x over previous
"""Trainium2 Bass kernel for nn_Chf_Likelihood_Loss.

Reference computes, for B=8 density maps of H=W=64:
    loss = mean_b sum_ij |CHF_ij(out_b) - CHF_ij(gt_b)|^2
where CHF_ij(m) = sum_n exp(I*(f_j*x_n + f_i*y_n)) m_n over the N=4096 pixels
and (f_i) are 2S=60 frequencies.

Algebraic reductions that make this tiny:
  1. CHF is linear in the map, so CHF(out) - CHF(gt) = CHF(out - gt).
  2. The angle f_j*x_w + f_i*y_h is separable, so the [60,60,4096] template
     contraction factorizes into skinny matmuls against [64,60] cos/sin
     factor matrices:
        A[i,w] = sum_h cos(f_i y_h) D[h,w],  Bm[i,w] = sum_h sin(f_i y_h) D[h,w]
        R = A@CxT - Bm@SxT,  I = A@SxT + Bm@CxT      (CxT[w,j] = cos(f_j x_w))
        loss_b = sum(R^2 + I^2)
  3. The subtraction in (1) is folded into the stage-1 matmul: stack d on
     partitions 0:64 and g on 64:128, with rhs = [trig; -trig], so the K=128
     contraction emits the transform of (d - g) directly.

Sharding: data-parallel over batch, one map per NeuronCore (B == 8 == n_cores).
Per core, two fp16 input DMAs (maps+stage-1 trig first, stage-2 trig second),
then: MM1 (K=128) -> two PSUM->SBUF casts -> MM2 (K=128) -> square on DVE ->
ones-matmul partition sum -> scalar reduce -> one-descriptor [1,1] DMA out.
Host sums the 8 per-core partials and divides by B.

Raw bacc (no TileContext): the Tile tail drain/EVSEM butterfly costs ~15us,
an order of magnitude more than this kernel's work, so semaphores are manual.
"""

import numpy as np

import concourse.bacc as bacc
import concourse.bass as bass
from concourse import mybir

B, H, W = 8, 64, 64
CHF_STEP = 30
CHF_TIK = 0.01
SAMPLE_STEP = 1.0
SCALE = 1.0
S2 = 2 * CHF_STEP  # 60 frequencies
N_CORES = 8

# Input arrives as two tensors so the stage-1 matmul can start as soon as the
# first (smaller) DMA lands:
#   blobA [128, 184]: [ map | T1 ]  with T1 rows 0:64 = [CT|ST], rows 64:128 = [-CT|-ST]
#   blobB [128, 121]: [ T2 | ones ] with T2 rows 0:64 = [CT|ST], rows 64:128 = [-ST|CT]
_C_M = 0
_C_T1 = W                 # stage-1 rhs (K=128, N=120) in blobA
_CA_END = W + 2 * S2      # 184
_C_T2 = 0                 # stage-2 rhs (K=128, N=120) in blobB
_C_ONE = 2 * S2           # ones column (partition-sum lhsT, K=60) in blobB
_CB_END = _C_ONE + 1      # 121

_F32 = mybir.dt.float32
# fp16 operands: single-pass PE matmuls (fp32 runs dual-pass LOW_HIGH), half
# the DMA bytes, fp32 PSUM accumulation. End-to-end rel err ~2e-5 (simulated).
_F16 = mybir.dt.float16


def _make_blob_consts() -> tuple[np.ndarray, np.ndarray]:
    """(t1, blobB) constant blocks.

    t1    [128, 120] = [[CT|ST]; [-CT|-ST]]
    blobB [128, 121] = [[CT|ST|ones]; [-ST|CT|ones]]
    with CT[w, j] = cos(f_j * x_w). x_axis == y_axis here (H == W, same
    sampling), so the same matrix serves the stage-1 (y) and stage-2 (x)
    contractions.
    """
    half = SAMPLE_STEP / 2
    x_axis = np.linspace(half, W * SAMPLE_STEP - half, W).astype(np.float32)
    freqs = (np.arange(-CHF_STEP, CHF_STEP) * CHF_TIK).astype(np.float32)
    ang = np.outer(x_axis, freqs).astype(np.float32)  # [W, S2]
    ct = np.cos(ang).astype(np.float32)
    st = np.sin(ang).astype(np.float32)
    ones = np.ones((W, 1), dtype=np.float32)
    t1 = np.concatenate(
        [np.concatenate([ct, st], axis=1), np.concatenate([-ct, -st], axis=1)], axis=0
    )
    blob_b = np.concatenate(
        [
            np.concatenate([ct, st, ones], axis=1),
            np.concatenate([-st, ct, ones], axis=1),
        ],
        axis=0,
    )
    return t1.astype(np.float16), np.ascontiguousarray(blob_b.astype(np.float16))


def _build_bass() -> bass.Bass:
    # Strip removable fixed overheads (~4.3us measured): the const-AP memsets
    # emitted in Bass.__init__ (this kernel never uses const APs) and the
    # bass-level all-engine barriers (init + Block exit). The data-dependency
    # semaphore chain below fully orders the kernel, and walrus's own NEFF
    # epilogue still drains + barriers every engine before the semaphore wipe.
    orig_barrier = bass.Bass.all_engine_barrier
    orig_memset = bass.BassGpSimd.memset
    bass.Bass.all_engine_barrier = lambda self, *a, **k: None
    bass.BassGpSimd.memset = lambda self, *a, **k: None
    try:
        nc = _build_bass_inner()
    finally:
        bass.Bass.all_engine_barrier = orig_barrier
        bass.BassGpSimd.memset = orig_memset
    return nc


def _build_bass_inner() -> bass.Bass:
    nc = bacc.Bacc("TRN2", target_bir_lowering=False, debug=False, num_devices=N_CORES)

    blob_a_in = nc.dram_tensor("blobA", [2 * H, _CA_END], _F16, kind="ExternalInput")
    blob_b_in = nc.dram_tensor("blobB", [2 * H, _CB_END], _F16, kind="ExternalInput")
    o_out = nc.dram_tensor("o", [1, 1], _F32, kind="ExternalOutput")

    with (
        nc.sbuf_tensor([2 * H, _CA_END], _F16) as blob_a,
        nc.sbuf_tensor([2 * H, _CB_END], _F16) as blob_b,
        nc.sbuf_tensor([2 * W, S2], _F16) as s1,
        nc.sbuf_tensor([S2, 2 * S2], _F16) as c2,
        nc.sbuf_tensor([S2, 2 * S2], _F16) as sq,
        nc.sbuf_tensor([1, 1], _F32) as acc,
        nc.psum_tensor([W, 2 * S2], _F32) as ps1,
        nc.psum_tensor([S2, 2 * S2], _F32) as ps2,
        nc.psum_tensor([1, 2 * S2], _F32) as ps3,
        nc.semaphore("dma_a") as dma_a_sem,
        nc.semaphore("dma_b") as dma_b_sem,
        nc.semaphore("pe") as pe_sem,
        nc.semaphore("dve") as dve_sem,
        nc.semaphore("dma_out") as dma_out_sem,
        nc.Block() as block,
    ):

        # The input DMAs issue from the Scalar/ACT sequencer (the second HWDGE
        # ring): it clears the walrus prologue ~0.9us before Sync, whose
        # prologue ends in a long DGE drain. A (map+T1) lands first and
        # unblocks MM1 while B (T2+ones) is still in flight.
        @block.scalar
        def _(scalar):
            scalar.dma_start(out=blob_a[:], in_=blob_a_in[:]).then_inc(dma_a_sem, 16)
            scalar.dma_start(out=blob_b[:], in_=blob_b_in[:]).then_inc(dma_b_sem, 16)

        @block.sync
        def _(sync):
            sync.wait_ge(dve_sem, 5)
            # [1,1] output: one descriptor; a [60,1] partition-strided store
            # costs ~7us in per-descriptor HBM-write latency. (Sync's ring
            # issues this in ~0.7us; a third DMA on the ACT ring took ~1.1us.)
            sync.dma_start(out=o_out[:], in_=acc[:]).then_inc(dma_out_sem, 16)
            # Hold the queue open until the output lands in HBM (NRT reads the
            # buffer as soon as all queues retire). Semaphores need no manual
            # reset: walrus's NEFF epilogue wipes the whole sem file for
            # re-execution.
            sync.wait_ge(dma_out_sem, 16)

        @block.tensor
        def _(tensor):
            # Stage 1 with the subtraction folded in:
            # ps1 = [d;g].T @ [trig;-trig] -> [W, 120] = [A^T | Bm^T] of (d-g)
            tensor.wait_ge(dma_a_sem, 16)
            nc.tensor.matmul(
                ps1[:],
                blob_a[:, _C_M : _C_M + W],
                blob_a[:, _C_T1 : _C_T1 + 2 * S2],
                start=True,
                stop=True,
            ).then_inc(pe_sem, 1)
            # blobB lands long before MM1 finishes; absorbing its wait here
            # keeps MM2 itself down to the single dve wait.
            tensor.wait_ge(dma_b_sem, 16)
            # Stage 2 as ONE K=128 matmul: lhsT = [A^T; Bm^T] stacked on
            # partitions, rhs = [[CT|ST]; [-ST|CT]], so
            # ps2 = [R | I] = A@[CT|ST] + Bm@[-ST|CT]
            tensor.wait_ge(dve_sem, 2)
            nc.tensor.matmul(
                ps2[:],
                s1[:],
                blob_b[:, _C_T2 : _C_T2 + 2 * S2],
                start=True,
                stop=True,
            ).then_inc(pe_sem, 1)
            # Partition sum of the squares: ps3 = ones.T @ sq -> [1, 120]
            tensor.wait_ge(dve_sem, 4)
            nc.tensor.matmul(
                ps3[:],
                blob_b[0:S2, _C_ONE : _C_ONE + 1],
                sq[:],
                start=True,
                stop=True,
            ).then_inc(pe_sem, 1)

        @block.vector
        def _(vector):
            # Matmul lhsT must live in SBUF; re-layout [A^T | Bm^T] [64,120]
            # into [A^T; Bm^T] [128,60] while bouncing ps1 out of PSUM.
            vector.wait_ge(pe_sem, 1)
            nc.vector.tensor_copy(s1[0:W, :], ps1[:, 0:S2]).then_inc(dve_sem, 1)
            nc.vector.tensor_copy(s1[W : 2 * W, :], ps1[:, S2 : 2 * S2]).then_inc(
                dve_sem, 1
            )
            # Square: sq = [R|I]^2 elementwise (one PSUM operand max per op)
            vector.wait_ge(pe_sem, 2)
            nc.vector.tensor_copy(c2[:], ps2[:]).then_inc(dve_sem, 1)
            vector.wait_ge(dve_sem, 3)
            nc.vector.tensor_mul(sq[:], c2[:], c2[:]).then_inc(dve_sem, 1)
            # Final free-axis reduce of the [1, 120] partition sums. ([1,1]
            # output: a [1,120] f32 store costs +330ns of DMA issue time.)
            vector.wait_ge(pe_sem, 3)
            nc.vector.reduce_sum(acc[:], ps3[:], axis=mybir.AxisListType.X).then_inc(
                dve_sem, 1
            )

    nc.compile()
    return nc


def _run(inputs: dict, trace: bool = False):
    from concourse.bass_utils import run_bass_kernel_spmd

    dnn = np.ascontiguousarray(np.asarray(inputs["dnn_output"], dtype=np.float32))
    gt = np.ascontiguousarray(np.asarray(inputs["gt_density_map"], dtype=np.float32))
    assert dnn.shape == (B, H, W) and gt.shape == (B, H, W)

    t1, blob_b = _make_blob_consts()
    nc = _build_bass()
    in_maps = []
    for b in range(B):
        maps = np.concatenate(
            [dnn[b].astype(np.float16), gt[b].astype(np.float16)], axis=0
        )  # [128, 64]
        blob_a = np.ascontiguousarray(np.concatenate([maps, t1], axis=1))
        in_maps.append({"blobA": blob_a, "blobB": blob_b})
    res = run_bass_kernel_spmd(nc, in_maps, list(range(N_CORES)), trace=trace)
    total = np.sum(
        np.stack([res.results[b]["o"] for b in range(B)]), dtype=np.float64
    )
    loss = np.float32(total / B * SCALE)
    return np.asarray(loss, dtype=np.float32), res


def kernel(**inputs) -> np.ndarray:
    loss, _ = _run(inputs, trace=False)
    return loss


# revision 59
# speedup vs baseline: 1.0277x; 1.0092x over previous
"""Trainium2 Bass kernel for nn_Chf_Likelihood_Loss.

Reference computes, for B=8 density maps of H=W=64:
    loss = mean_b sum_ij |CHF_ij(out_b) - CHF_ij(gt_b)|^2
where CHF_ij(m) = sum_n exp(I*(f_j*x_n + f_i*y_n)) m_n over the N=4096 pixels
and (f_i) are 2S=60 frequencies.

Algebraic reductions that make this tiny:
  1. CHF is linear in the map, so CHF(out) - CHF(gt) = CHF(out - gt).
  2. The angle f_j*x_w + f_i*y_h is separable, so the [60,60,4096] template
     contraction factorizes into skinny matmuls against [64,60] cos/sin
     factor matrices:
        A[i,w] = sum_h cos(f_i y_h) D[h,w],  Bm[i,w] = sum_h sin(f_i y_h) D[h,w]
        R = A@CxT - Bm@SxT,  I = A@SxT + Bm@CxT      (CxT[w,j] = cos(f_j x_w))
        loss_b = sum(R^2 + I^2)
  3. The subtraction in (1) is folded into the stage-1 matmul: stack d on
     partitions 0:64 and g on 64:128, with rhs = [trig; -trig], so the K=128
     contraction emits the transform of (d - g) directly.

Sharding: data-parallel over batch, one map per NeuronCore (B == 8 == n_cores).
Per core, two fp16 input DMAs (maps+stage-1 trig first, stage-2 trig second),
then: MM1 (K=128) -> two PSUM->SBUF casts -> MM2 (K=128) -> square on DVE ->
ones-matmul partition sum -> scalar reduce -> one-descriptor [1,1] DMA out.
Host sums the 8 per-core partials and divides by B.

Raw bacc (no TileContext): the Tile tail drain/EVSEM butterfly costs ~15us,
an order of magnitude more than this kernel's work, so semaphores are manual.
"""

import numpy as np

import concourse.bacc as bacc
import concourse.bass as bass
from concourse import mybir

B, H, W = 8, 64, 64
CHF_STEP = 30
CHF_TIK = 0.01
SAMPLE_STEP = 1.0
SCALE = 1.0
S2 = 2 * CHF_STEP  # 60 frequencies
N_CORES = 8

# Input arrives as two tensors so the stage-1 matmul can start as soon as the
# first (smaller) DMA lands:
#   blobA [128, 184]: [ map | T1 ]  with T1 rows 0:64 = [CT|ST], rows 64:128 = [-CT|-ST]
#   blobB [128, 121]: [ T2 | ones ] with T2 rows 0:64 = [CT|ST], rows 64:128 = [-ST|CT]
_C_M = 0
_C_T1 = W                 # stage-1 rhs (K=128, N=120) in blobA
_CA_END = W + 2 * S2      # 184
_C_T2 = 0                 # stage-2 rhs (K=128, N=120) in blobB
_C_ONE = 2 * S2           # ones column (partition-sum lhsT, K=60) in blobB
_CB_END = _C_ONE + 1      # 121

_F32 = mybir.dt.float32
# fp16 operands: single-pass PE matmuls (fp32 runs dual-pass LOW_HIGH), half
# the DMA bytes, fp32 PSUM accumulation. End-to-end rel err ~2e-5 (simulated).
_F16 = mybir.dt.float16


def _make_blob_consts() -> tuple[np.ndarray, np.ndarray]:
    """(t1, blobB) constant blocks.

    t1    [128, 120] = [[CT|ST]; [-CT|-ST]]
    blobB [128, 121] = [[CT|ST|ones]; [-ST|CT|ones]]
    with CT[w, j] = cos(f_j * x_w). x_axis == y_axis here (H == W, same
    sampling), so the same matrix serves the stage-1 (y) and stage-2 (x)
    contractions.
    """
    half = SAMPLE_STEP / 2
    x_axis = np.linspace(half, W * SAMPLE_STEP - half, W).astype(np.float32)
    freqs = (np.arange(-CHF_STEP, CHF_STEP) * CHF_TIK).astype(np.float32)
    ang = np.outer(x_axis, freqs).astype(np.float32)  # [W, S2]
    ct = np.cos(ang).astype(np.float32)
    st = np.sin(ang).astype(np.float32)
    ones = np.ones((W, 1), dtype=np.float32)
    t1 = np.concatenate(
        [np.concatenate([ct, st], axis=1), np.concatenate([-ct, -st], axis=1)], axis=0
    )
    blob_b = np.concatenate(
        [
            np.concatenate([ct, st, ones], axis=1),
            np.concatenate([-st, ct, ones], axis=1),
        ],
        axis=0,
    )
    return t1.astype(np.float16), np.ascontiguousarray(blob_b.astype(np.float16))


def _build_bass() -> bass.Bass:
    # Strip removable fixed overheads (~4.3us measured): the const-AP memsets
    # emitted in Bass.__init__ (this kernel never uses const APs) and the
    # bass-level all-engine barriers (init + Block exit). The data-dependency
    # semaphore chain below fully orders the kernel, and walrus's own NEFF
    # epilogue still drains + barriers every engine before the semaphore wipe.
    orig_barrier = bass.Bass.all_engine_barrier
    orig_memset = bass.BassGpSimd.memset
    bass.Bass.all_engine_barrier = lambda self, *a, **k: None
    bass.BassGpSimd.memset = lambda self, *a, **k: None
    try:
        nc = _build_bass_inner()
    finally:
        bass.Bass.all_engine_barrier = orig_barrier
        bass.BassGpSimd.memset = orig_memset
    return nc


def _build_bass_inner() -> bass.Bass:
    nc = bacc.Bacc("TRN2", target_bir_lowering=False, debug=False, num_devices=N_CORES)

    blob_a_in = nc.dram_tensor("blobA", [2 * H, _CA_END], _F16, kind="ExternalInput")
    blob_b_in = nc.dram_tensor("blobB", [2 * H, _CB_END], _F16, kind="ExternalInput")
    o_out = nc.dram_tensor("o", [1, 1], _F32, kind="ExternalOutput")

    with (
        nc.sbuf_tensor([2 * H, _CA_END], _F16) as blob_a,
        nc.sbuf_tensor([2 * H, _CB_END], _F16) as blob_b,
        nc.sbuf_tensor([2 * W, S2], _F16) as s1,
        nc.sbuf_tensor([S2, 2 * S2], _F16) as c2,
        nc.sbuf_tensor([S2, 2 * S2], _F16) as sq,
        nc.sbuf_tensor([1, 1], _F32) as acc,
        nc.sbuf_tensor([1, 1], _F16) as warm,
        nc.psum_tensor([W, 2 * S2], _F32) as ps1,
        nc.psum_tensor([S2, 2 * S2], _F32) as ps2,
        nc.psum_tensor([1, 2 * S2], _F32) as ps3,
        nc.semaphore("dma_a") as dma_a_sem,
        nc.semaphore("dma_b") as dma_b_sem,
        nc.semaphore("pe") as pe_sem,
        nc.semaphore("dve") as dve_sem,
        nc.semaphore("dma_out") as dma_out_sem,
        nc.Block() as block,
    ):

        # The input DMAs issue from the Scalar/ACT sequencer (the second HWDGE
        # ring): it clears the walrus prologue ~0.9us before Sync, whose
        # prologue ends in a long DGE drain. A (map+T1) lands first and
        # unblocks MM1 while B (T2+ones) is still in flight.
        @block.scalar
        def _(scalar):
            scalar.dma_start(out=blob_a[:], in_=blob_a_in[:]).then_inc(dma_a_sem, 16)
            scalar.dma_start(out=blob_b[:], in_=blob_b_in[:]).then_inc(dma_b_sem, 16)

        @block.sync
        def _(sync):
            # Warm-up on Sync's otherwise-idle HWDGE ring during the input-DMA
            # wait: the first DMA on a ring pays ~120ns extra issue cost, so
            # spend it here instead of on the output.
            sync.dma_start(out=warm[:], in_=blob_a_in[0:1, 0:1]).then_inc(
                dma_out_sem, 16
            )
            sync.wait_ge(dve_sem, 5)
            # [1,1] output: one descriptor; a [60,1] partition-strided store
            # costs ~7us in per-descriptor HBM-write latency. (Sync's ring
            # issues this in ~0.7us; a third DMA on the ACT ring took ~1.1us.)
            sync.dma_start(out=o_out[:], in_=acc[:]).then_inc(dma_out_sem, 16)
            # Hold the queue open until the output lands in HBM (NRT reads the
            # buffer as soon as all queues retire; >=32 covers warm-up +
            # output). Semaphores need no manual reset: walrus's NEFF epilogue
            # wipes the whole sem file for re-execution.
            sync.wait_ge(dma_out_sem, 32)

        @block.tensor
        def _(tensor):
            # Stage 1 with the subtraction folded in:
            # ps1 = [d;g].T @ [trig;-trig] -> [W, 120] = [A^T | Bm^T] of (d-g)
            tensor.wait_ge(dma_a_sem, 16)
            nc.tensor.matmul(
                ps1[:],
                blob_a[:, _C_M : _C_M + W],
                blob_a[:, _C_T1 : _C_T1 + 2 * S2],
                start=True,
                stop=True,
            ).then_inc(pe_sem, 1)
            # blobB lands long before MM1 finishes; absorbing its wait here
            # keeps MM2 itself down to the single dve wait.
            tensor.wait_ge(dma_b_sem, 16)
            # Stage 2 as ONE K=128 matmul: lhsT = [A^T; Bm^T] stacked on
            # partitions, rhs = [[CT|ST]; [-ST|CT]], so
            # ps2 = [R | I] = A@[CT|ST] + Bm@[-ST|CT]
            tensor.wait_ge(dve_sem, 2)
            nc.tensor.matmul(
                ps2[:],
                s1[:],
                blob_b[:, _C_T2 : _C_T2 + 2 * S2],
                start=True,
                stop=True,
            ).then_inc(pe_sem, 1)
            # Partition sum of the squares: ps3 = ones.T @ sq -> [1, 120]
            tensor.wait_ge(dve_sem, 4)
            nc.tensor.matmul(
                ps3[:],
                blob_b[0:S2, _C_ONE : _C_ONE + 1],
                sq[:],
                start=True,
                stop=True,
            ).then_inc(pe_sem, 1)

        @block.vector
        def _(vector):
            # Matmul lhsT must live in SBUF; re-layout [A^T | Bm^T] [64,120]
            # into [A^T; Bm^T] [128,60] while bouncing ps1 out of PSUM.
            vector.wait_ge(pe_sem, 1)
            nc.vector.tensor_copy(s1[0:W, :], ps1[:, 0:S2]).then_inc(dve_sem, 1)
            nc.vector.tensor_copy(s1[W : 2 * W, :], ps1[:, S2 : 2 * S2]).then_inc(
                dve_sem, 1
            )
            # Square: sq = [R|I]^2 elementwise (one PSUM operand max per op)
            vector.wait_ge(pe_sem, 2)
            nc.vector.tensor_copy(c2[:], ps2[:]).then_inc(dve_sem, 1)
            vector.wait_ge(dve_sem, 3)
            nc.vector.tensor_mul(sq[:], c2[:], c2[:]).then_inc(dve_sem, 1)
            # Final free-axis reduce of the [1, 120] partition sums. ([1,1]
            # output: a [1,120] f32 store costs +330ns of DMA issue time.)
            vector.wait_ge(pe_sem, 3)
            nc.vector.reduce_sum(acc[:], ps3[:], axis=mybir.AxisListType.X).then_inc(
                dve_sem, 1
            )

    nc.compile()
    return nc


def _run(inputs: dict, trace: bool = False):
    from concourse.bass_utils import run_bass_kernel_spmd

    dnn = np.ascontiguousarray(np.asarray(inputs["dnn_output"], dtype=np.float32))
    gt = np.ascontiguousarray(np.asarray(inputs["gt_density_map"], dtype=np.float32))
    assert dnn.shape == (B, H, W) and gt.shape == (B, H, W)

    t1, blob_b = _make_blob_consts()
    nc = _build_bass()
    in_maps = []
    for b in range(B):
        maps = np.concatenate(
            [dnn[b].astype(np.float16), gt[b].astype(np.float16)], axis=0
        )  # [128, 64]
        blob_a = np.ascontiguousarray(np.concatenate([maps, t1], axis=1))
        in_maps.append({"blobA": blob_a, "blobB": blob_b})
    res = run_bass_kernel_spmd(nc, in_maps, list(range(N_CORES)), trace=trace)
    total = np.sum(
        np.stack([res.results[b]["o"] for b in range(B)]), dtype=np.float64
    )
    loss = np.float32(total / B * SCALE)
    return np.asarray(loss, dtype=np.float32), res


def kernel(**inputs) -> np.ndarray:
    loss, _ = _run(inputs, trace=False)
    return loss
